# revision 2
# baseline (speedup 1.0000x reference)
"""TensorProductConvLayer (DiffDock) Bass kernel for 8 Trainium2 cores, v3.

Metric = warm wall-clock of kernel(); the axon link moves ~70MB/s in and
~25MB/s out, so the design minimizes bytes over the link and keeps host
numpy work small. See probe.py for primitive validation.

Layout:
  - Global 128-node windows w = src>>7 (782 real). Core c owns windows
    [98c, 98c+98) = output nodes [12544c, 12544c+12544). Each window gets a
    static capacity of PW=1536 edge slots (seed-0 max occupancy is 1385);
    host scatters edges (any order within a window) into their window's
    slot range, pad slots have sh=0 so their tp contribution is exactly 0.
  - Device, per 512-edge block: MLP (edge_attr -> 48 -> 320 per-edge TP
    weights) on the PE with features on partitions; destination-node
    features fetched by indirect DMA from a row-replicated node table
    (built on device after an AllGather of 1/8 node slices) and PE-
    transposed straight into the replicated layout; TP via DVE elementwise
    + sparse stationary matmul reduction; spherical harmonics applied via
    small replicate matmuls.
  - Segment-sum: per 128-edge tile build a one-hot matrix
    sel[e, n] = (src[e]&127 == n) with a DVE is_equal against an iota
    constant, then accumulate sel^T @ tp into the window's PSUM tile over
    the window's 12 tiles; on the last tile convert to bf16 and DMA the
    128 output rows. Output = mean after a host-side divide by counts.
"""

import numpy as np
import ml_dtypes

bfl = ml_dtypes.bfloat16
f8np = ml_dtypes.float8_e4m3fn

E_TOT = 1_000_000
N_NODES = 100_000
NCORES = 8
NS = 16
NW = 98                  # windows per core
WN = 128                 # nodes per window
PW = 1536                # edge slots per window (12 tiles of 128)
TPW = PW // 128          # 12 tiles per window
NPADC = NW * WN          # 12544 output rows per core
EP = NW * PW             # 150528 edge slots per core
BLK = 512
NB = EP // BLK           # 294
NTOT = NPADC * NCORES    # 100352 rows in allgathered node table
EA_INT8 = True

_CACHE = {}
LAST_RESULTS = None


class _Results:
    """Shim mirroring BassKernelResults for the test harness."""

    def __init__(self, results):
        self.results = results
        self.exec_time_ns = None
        self.instructions_and_trace = None
        self.profile_json = None


def _get_runner(nc):
    """Build (once) a cached jitted SPMD executor for nc.

    Same execution mechanism run_bass_kernel_spmd uses under axon
    (bass2jax._bass_exec_p -> bass_exec custom call -> NEFF via PJRT on the
    8 cores), but the jitted callable, mesh, and name lists are built once
    and reused, and per-core inputs are passed as already-concatenated
    global arrays so no per-call retrace or concat copies happen.
    """
    import jax
    import numpy as _np
    from jax.experimental.shard_map import shard_map
    from jax.sharding import Mesh, PartitionSpec
    from concourse import bass2jax, mybir

    bass2jax.install_neuronx_cc_hook()
    in_names, out_names, out_avals, zero_shapes = [], [], [], []
    for alloc in nc.m.functions[0].allocations:
        if not isinstance(alloc, mybir.MemoryLocationSet):
            continue
        name = alloc.memorylocations[0].name
        if alloc.kind == "ExternalInput":
            in_names.append(name)
        elif alloc.kind == "ExternalOutput":
            out_names.append(name)
            shape = tuple(alloc.tensor_shape)
            dtype = mybir.dt.np(alloc.dtype)
            out_avals.append(jax.core.ShapedArray(shape, dtype))
            zero_shapes.append((shape, dtype))
    n_params = len(in_names)
    n_outs = len(out_names)
    names_all = tuple(in_names) + tuple(out_names)

    def _body(*args):
        outs = bass2jax._bass_exec_p.bind(
            *args,
            out_avals=tuple(out_avals),
            in_names=names_all,
            out_names=tuple(out_names),
            lowering_input_output_aliases=(),
            sim_require_finite=True,
            sim_require_nnan=True,
            nc=nc,
        )
        return tuple(outs)

    devices = jax.devices()[:NCORES]
    mesh = Mesh(_np.asarray(devices), ("core",))
    in_specs = (PartitionSpec("core"),) * (n_params + n_outs)
    out_specs = (PartitionSpec("core"),) * n_outs
    donate = tuple(range(n_params, n_params + n_outs))
    jitted = jax.jit(
        shard_map(_body, mesh=mesh, in_specs=in_specs, out_specs=out_specs,
                  check_rep=False),
        donate_argnums=donate, keep_unused=True)

    def run(global_ins: dict):
        zeros = [_np.zeros((NCORES * s[0], *s[1:]), dt)
                 for s, dt in zero_shapes]
        out_arrs = jitted(*[global_ins[n] for n in in_names], *zeros)
        results = [
            {name: _np.asarray(out_arrs[i]).reshape(
                NCORES, *zero_shapes[i][0])[c] for i, name in enumerate(out_names)}
            for c in range(NCORES)
        ]
        return _Results(results)

    return run


def _build_bass():
    import concourse.bass as bass
    import concourse.bacc as bacc
    import concourse.mybir as mybir
    import concourse.tile as tile
    from concourse.masks import make_identity

    f32 = mybir.dt.float32
    bf16 = mybir.dt.bfloat16
    i32 = mybir.dt.int32
    f16 = mybir.dt.float16
    AF = mybir.ActivationFunctionType
    ea_dt = mybir.dt.uint8 if EA_INT8 else bf16

    nc = bacc.Bacc(None, target_bir_lowering=False, enable_partition_id=False,
                   num_devices=NCORES)
    ea = nc.dram_tensor("ea", [EP, 48], ea_dt, kind="ExternalInput")
    shT = nc.dram_tensor("shT", [4, EP], bf16, kind="ExternalInput")
    dstv = nc.dram_tensor("dstv", [EP, 1], i32, kind="ExternalInput")
    srcw = nc.dram_tensor("srcw", [EP, 1], bf16, kind="ExternalInput")
    nodes = nc.dram_tensor("nodes", [NPADC, 16], f16, kind="ExternalInput")
    w1d = nc.dram_tensor("w1d", [48, 48], f32, kind="ExternalInput")
    b1d = nc.dram_tensor("b1d", [48, 1], f32, kind="ExternalInput")
    w2d = nc.dram_tensor("w2d", [48, 320], f32, kind="ExternalInput")
    r16a = nc.dram_tensor("r16a", [128, 20], f32, kind="ExternalInput")
    r16b = nc.dram_tensor("r16b", [128, 20], f32, kind="ExternalInput")
    r4p = nc.dram_tensor("r4p", [64, 20], f32, kind="ExternalInput")
    rs0 = nc.dram_tensor("rs0", [4, 16], bf16, kind="ExternalInput")
    rs3 = nc.dram_tensor("rs3", [4, 12], bf16, kind="ExternalInput")
    rqd = nc.dram_tensor("rqd", [20, 12], f32, kind="ExternalInput")
    iot = nc.dram_tensor("iot", [128, 128], bf16, kind="ExternalInput")

    outp = nc.dram_tensor("outp", [NPADC, 28], bf16, kind="ExternalOutput")

    nodes_b = nc.dram_tensor("nodes_b", [NPADC, 16], f16)
    nodes_full = nc.dram_tensor("nodes_full", [NTOT, 16], f16,
                                addr_space="Shared")
    nrep = nc.dram_tensor("nrep", [NTOT, 128], f16)

    AP = bass.AP

    def dram_ap(t, off, dims):
        return AP(t, off, [list(d) for d in dims])

    with tile.TileContext(nc) as tc:
        with tc.tile_pool(name="init", bufs=2) as ip:
            nc.sync.dma_start(out=nodes_b[:, :], in_=nodes[:, :])
            nc.gpsimd.collective_compute(
                "AllGather", mybir.AluOpType.bypass,
                replica_groups=[list(range(NCORES))],
                ins=[nodes_b[:].opt()],
                outs=[nodes_full[:].opt()],
            )
            # row-replicated gather table nrep[n] = tile(x[n], 8)
            for i in range(NTOT // 1024):
                tin = ip.tile([128, 8, 16], f16, tag="tin")
                nc.sync.dma_start(
                    out=tin[:],
                    in_=dram_ap(nodes_full, 1024 * i * 16,
                                [[16, 128], [2048, 8], [1, 16]]))
                a = tin[:]
                brd = AP(a.tensor, a.offset,
                         [list(a.ap[0]), list(a.ap[1]), [0, 8], list(a.ap[2])])
                rep = ip.tile([128, 8, 8, 16], f16, tag="rep")
                nc.vector.tensor_copy(out=rep[:], in_=brd)
                nc.sync.dma_start(
                    out=dram_ap(nrep, 1024 * i * 128,
                                [[128, 128], [16384, 8], [16, 8], [1, 16]]),
                    in_=rep[:])

        with (
            tc.tile_pool(name="const", bufs=1) as cp,
            tc.tile_pool(name="sb", bufs=3) as sb,
            tc.tile_pool(name="ps", bufs=1, space="PSUM") as pp,
            tc.tile_pool(name="ps2", bufs=1, space="PSUM") as pp2,
            tc.tile_pool(name="psw", bufs=2, space="PSUM") as pw_pool,
        ):
            idn = cp.tile([128, 128], f32)
            make_identity(nc, idn[:])
            iota_sb = cp.tile([128, 128], bf16)
            nc.sync.dma_start(out=iota_sb[:], in_=iot[:, :])
            w1_sb = cp.tile([48, 48], f32)
            nc.sync.dma_start(out=w1_sb[:], in_=w1d[:, :])
            b1_sb = cp.tile([48, 1], f32)
            nc.sync.dma_start(out=b1_sb[:], in_=b1d[:, :])
            w2_sb = cp.tile([48, 320], f32)
            nc.sync.dma_start(out=w2_sb[:], in_=w2d[:, :])
            r16a_sb = cp.tile([128, 20], f32)
            nc.sync.dma_start(out=r16a_sb[:], in_=r16a[:, :])
            r16b_sb = cp.tile([128, 20], f32)
            nc.sync.dma_start(out=r16b_sb[:], in_=r16b[:, :])
            r4p_sb = cp.tile([64, 20], f32)
            nc.sync.dma_start(out=r4p_sb[:], in_=r4p[:, :])
            rs0_sb = cp.tile([4, 16], bf16)
            nc.sync.dma_start(out=rs0_sb[:], in_=rs0[:, :])
            rs3_sb = cp.tile([4, 12], bf16)
            nc.sync.dma_start(out=rs3_sb[:], in_=rs3[:, :])
            rq_sb = cp.tile([20, 12], f32)
            nc.sync.dma_start(out=rq_sb[:], in_=rqd[:, :])

            win_ps = None
            for b in range(NB):
                # ---- load ea block [128, 4, 48] ----
                ea8_sb = sb.tile([128, 4, 48],
                                 mybir.dt.uint8 if EA_INT8 else bf16, tag="ea8")
                nc.sync.dma_start(
                    out=ea8_sb[:],
                    in_=dram_ap(ea, 512 * b * 48,
                                [[48, 128], [6144, 4], [1, 48]]))
                ea_sb = sb.tile([128, 4, 48], f32, tag="ea")
                nc.scalar.activation(ea_sb[:], ea8_sb[:], AF.Copy,
                                     bias=-128.0)

                # ---- transpose to eaT [48, 512] ----
                tr_ps = pp.tile([128, 512], f32, tag="tr")
                for c in range(4):
                    nc.tensor.transpose(out=tr_ps[0:48, 128 * c:128 * (c + 1)],
                                        in_=ea_sb[:, c, :], identity=idn[:])
                eaT_sb = sb.tile([48, 512], f32, tag="eaT")
                nc.scalar.activation(eaT_sb[:], tr_ps[0:48, :], AF.Copy)

                # ---- MLP ----
                ph_ps = pp.tile([48, 512], f32, tag="ph")
                nc.tensor.matmul(ph_ps[:], lhsT=w1_sb[:], rhs=eaT_sb[:],
                                 start=True, stop=True)
                h_sb = sb.tile([48, 512], f32, tag="h")
                nc.scalar.activation(h_sb[:], ph_ps[:], AF.Relu,
                                     bias=b1_sb[:, 0:1])
                pc_ps = pp2.tile([128, 1536], f32, tag="pc")
                nc.tensor.matmul(pc_ps[0:128, 0:512], lhsT=w2_sb[:, 0:128],
                                 rhs=h_sb[:], start=True, stop=True)
                nc.tensor.matmul(pc_ps[0:128, 512:1024], lhsT=w2_sb[:, 128:256],
                                 rhs=h_sb[:], start=True, stop=True)
                nc.tensor.matmul(pc_ps[0:64, 1024:1536], lhsT=w2_sb[:, 256:320],
                                 rhs=h_sb[:], start=True, stop=True)

                # ---- gather replicated x rows, transpose to Xrep [128,512] ----
                dst_sb = sb.tile([128, 4], i32, tag="dst")
                nc.sync.dma_start(
                    out=dst_sb[:],
                    in_=dram_ap(dstv, 512 * b, [[1, 128], [128, 4]]))
                xg_sb = sb.tile([128, 4, 128], f16, tag="xg")
                for c in range(4):
                    nc.gpsimd.indirect_dma_start(
                        out=xg_sb[:, c, :], out_offset=None,
                        in_=nrep[:],
                        in_offset=bass.IndirectOffsetOnAxis(
                            ap=dst_sb[:, c:c + 1], axis=0),
                    )
                xgf_sb = sb.tile([128, 4, 128], f32, tag="xgf")
                nc.scalar.activation(xgf_sb[:], xg_sb[:], AF.Copy)
                for c in range(4):
                    nc.tensor.transpose(out=tr_ps[:, 128 * c:128 * (c + 1)],
                                        in_=xgf_sb[:, c, :], identity=idn[:])
                xr_sb = sb.tile([128, 512], f32, tag="xr")
                nc.scalar.activation(xr_sb[:], tr_ps[:], AF.Copy)

                # ---- TP elementwise + i-reduction ----
                c1_sb = sb.tile([128, 512], f32, tag="c1")
                nc.vector.tensor_tensor(out=c1_sb[:], in0=xr_sb[:],
                                        in1=pc_ps[0:128, 0:512],
                                        op=mybir.AluOpType.mult)
                c2_sb = sb.tile([128, 512], f32, tag="c2")
                nc.vector.tensor_tensor(out=c2_sb[:], in0=xr_sb[:],
                                        in1=pc_ps[0:128, 512:1024],
                                        op=mybir.AluOpType.mult)
                c3_sb = sb.tile([64, 512], f32, tag="c3")
                nc.vector.tensor_tensor(out=c3_sb[:], in0=xr_sb[0:64, :],
                                        in1=pc_ps[0:64, 1024:1536],
                                        op=mybir.AluOpType.mult)
                mix_ps = pp.tile([128, 512], f32, tag="mix")
                po = mix_ps[0:20, :]
                nc.tensor.matmul(po, lhsT=r16a_sb[:], rhs=c1_sb[:],
                                 start=True, stop=False)
                nc.tensor.matmul(po, lhsT=r16b_sb[:], rhs=c2_sb[:],
                                 start=False, stop=False)
                nc.tensor.matmul(po, lhsT=r4p_sb[:], rhs=c3_sb[:],
                                 start=False, stop=True)
                po_sb = sb.tile([20, 512], f32, tag="posb")
                nc.scalar.activation(po_sb[:], po, AF.Copy)

                # ---- spherical harmonics ----
                shT_sb = sb.tile([4, 512], bf16, tag="sh")
                nc.sync.dma_start(
                    out=shT_sb[:],
                    in_=dram_ap(shT, 512 * b, [[EP, 4], [1, 512]]))
                nc.tensor.matmul(mix_ps[32:48, :], lhsT=rs0_sb[:],
                                 rhs=shT_sb[:], start=True, stop=True)
                nc.tensor.matmul(mix_ps[64:76, :], lhsT=rs3_sb[:],
                                 rhs=shT_sb[:], start=True, stop=True)
                nc.tensor.matmul(ph_ps[0:12, :], lhsT=rq_sb[:],
                                 rhs=po_sb[:], start=True, stop=True)
                sh12_sb = sb.tile([12, 512], f32, tag="sh12")
                nc.scalar.activation(sh12_sb[:], mix_ps[64:76, :], AF.Copy)
                tpt_sb = sb.tile([16, 512], f32, tag="tpt")
                nc.vector.tensor_tensor(out=tpt_sb[:], in0=po_sb[0:16, :],
                                        in1=mix_ps[32:48, :],
                                        op=mybir.AluOpType.mult)
                tpb_sb = sb.tile([12, 512], f32, tag="tpb")
                nc.vector.tensor_tensor(out=tpb_sb[:], in0=sh12_sb[:],
                                        in1=ph_ps[0:12, :],
                                        op=mybir.AluOpType.mult)

                # ---- transpose tp to edge-major ----
                for c in range(4):
                    nc.tensor.transpose(out=tr_ps[:, 128 * c:128 * c + 16],
                                        in_=tpt_sb[:, 128 * c:128 * (c + 1)],
                                        identity=idn[0:16, 0:16])
                    nc.tensor.transpose(out=tr_ps[:, 128 * c + 16:128 * c + 28],
                                        in_=tpb_sb[:, 128 * c:128 * (c + 1)],
                                        identity=idn[0:12, 0:12])
                tpe_sb = sb.tile([128, 4, 28], f32, tag="tpe")
                for c in range(4):
                    nc.scalar.activation(tpe_sb[:, c, :],
                                         tr_ps[:, 128 * c:128 * c + 28],
                                         AF.Copy)

                # ---- windowed one-hot scatter ----
                srcw_sb = sb.tile([128, 4], bf16, tag="srcw")
                nc.sync.dma_start(
                    out=srcw_sb[:],
                    in_=dram_ap(srcw, 512 * b, [[1, 128], [128, 4]]))
                for c in range(4):
                    h = 4 * b + c
                    w, hw = divmod(h, TPW)
                    if hw == 0:
                        win_ps = pw_pool.tile([128, 28], f32, tag="win")
                    sel_sb = sb.tile([128, 128], f32, tag="sel")
                    nc.vector.tensor_tensor(
                        out=sel_sb[:],
                        in0=srcw_sb[:, c:c + 1].to_broadcast([128, 128]),
                        in1=iota_sb[:],
                        op=mybir.AluOpType.is_equal)
                    nc.tensor.matmul(win_ps[:], lhsT=sel_sb[:],
                                     rhs=tpe_sb[:, c, :],
                                     start=(hw == 0), stop=(hw == TPW - 1))
                    if hw == TPW - 1:
                        o_sb = sb.tile([128, 28], bf16, tag="ob")
                        nc.scalar.activation(o_sb[:], win_ps[:], AF.Copy)
                        nc.sync.dma_start(
                            out=dram_ap(outp, 128 * w * 28,
                                        [[28, 128], [1, 28]]),
                            in_=o_sb[:])
    nc.finalize()
    return nc


def _prep_consts(w1, b1, w2, b2, ea_scale):
    inv = np.float32(1.0 / np.sqrt(np.float32(NS)))
    w1 = np.asarray(w1, np.float32)
    b1 = np.asarray(b1, np.float32)
    w2 = np.asarray(w2, np.float32)
    b2 = np.asarray(b2, np.float32)
    assert not np.any(b2), "nonzero b2 unsupported"
    wb = w2 * inv
    p = np.arange(256)
    perm0 = (p % 16) * 16 + p // 16            # row 16j+i <- col i*16+j
    p = np.arange(64)
    perm1 = 256 + (p % 16) * 4 + p // 16       # row 16u+i <- col 256+i*4+u
    w2c = np.ascontiguousarray(wb[:, np.concatenate([perm0, perm1])])

    r16a = np.zeros((128, 20), np.float32)
    r16a[np.arange(128), np.arange(128) // 16] = 1.0
    r16b = np.zeros((128, 20), np.float32)
    r16b[np.arange(128), 8 + np.arange(128) // 16] = 1.0
    r4p = np.zeros((64, 20), np.float32)
    r4p[np.arange(64), 16 + np.arange(64) // 16] = 1.0
    rs0 = np.zeros((4, 16), np.float32)
    rs0[0, :] = 1.0
    rs3 = np.zeros((4, 12), np.float32)
    rq = np.zeros((20, 12), np.float32)
    for u in range(4):
        for m in range(3):
            rs3[1 + m, 3 * u + m] = 1.0
            rq[16 + u, 3 * u + m] = 1.0
    iota = np.broadcast_to(np.arange(128, dtype=np.float32), (128, 128))
    w1s = np.ascontiguousarray(w1 / ea_scale)
    return {"w1d": w1s, "b1d": b1.reshape(48, 1).astype(np.float32),
            "w2d": w2c, "r16a": r16a, "r16b": r16b,
            "r4p": r4p, "rs0": rs0.astype(bfl),
            "rs3": rs3.astype(bfl), "rqd": rq,
            "iot": np.ascontiguousarray(iota).astype(bfl)}


def kernel(node_attr, edge_index, edge_attr, edge_sh, w1, b1, w2, b2):
    global LAST_RESULTS
    from concourse.bass_utils import run_bass_kernel_spmd

    src = np.asarray(edge_index[0]).astype(np.int32, copy=False)
    dst = np.asarray(edge_index[1]).astype(np.int32, copy=False)
    edge_attr = np.asarray(edge_attr, np.float32)
    edge_sh = np.asarray(edge_sh, np.float32)
    node_attr = np.asarray(node_attr, np.float32)

    ea_scale = np.float32(127.0) / np.float32(np.abs(edge_attr).max())
    consts = _prep_consts(w1, b1, w2, b2, ea_scale)

    wg = src >> 7                                  # global window id
    order = np.argsort(wg, kind="stable")
    wcnt = np.bincount(wg, minlength=NW * NCORES)
    assert wcnt.max() <= PW, f"window overflow: {wcnt.max()} > {PW}"
    wstart = np.zeros(NW * NCORES + 1, np.int64)
    wstart[1:] = np.cumsum(wcnt)
    ws = wg[order]
    rank = np.arange(E_TOT, dtype=np.int64) - wstart[ws]
    slot = (ws // NW) * EP + (ws % NW) * PW + rank   # into [8*EP]

    ea_all = (edge_attr * ea_scale + np.float32(128.5)).astype(np.uint8)
    ea_pad = np.zeros((NCORES * EP, 48), np.uint8)
    ea_pad[slot] = ea_all[order]
    sh_pad = np.zeros((NCORES * EP, 4), bfl)
    sh_pad[slot] = edge_sh[:, :4].astype(bfl)[order]
    shT_pad = np.ascontiguousarray(
        sh_pad.reshape(NCORES, EP, 4).transpose(0, 2, 1))
    dst_pad = np.zeros((NCORES * EP, 1), np.int32)
    dst_pad[slot, 0] = dst[order]
    srcw_pad = np.zeros((NCORES * EP, 1), bfl)
    srcw_pad[slot, 0] = (src[order] & 127).astype(bfl)
    nodes_pad = np.zeros((NCORES, NPADC, 16), np.float16)
    flat = nodes_pad.reshape(NCORES * NPADC, 16)
    flat[:N_NODES] = node_attr.astype(np.float16)

    # global (concatenated-over-cores) layout for the cached SPMD runner
    global_ins = {
        "ea": ea_pad,
        "shT": shT_pad.reshape(NCORES * 4, EP),
        "dstv": dst_pad,
        "srcw": srcw_pad,
        "nodes": nodes_pad.reshape(NCORES * NPADC, 16),
    }
    for k, v in consts.items():
        global_ins[k] = np.ascontiguousarray(
            np.broadcast_to(v, (NCORES, *v.shape))).reshape(
                NCORES * v.shape[0], *v.shape[1:])

    if "nc" not in _CACHE:
        _CACHE["nc"] = _build_bass()
    if "runner" not in _CACHE:
        _CACHE["runner"] = _get_runner(_CACHE["nc"])

    res = _CACHE["runner"](global_ins)
    LAST_RESULTS = res

    big = np.empty((NCORES * NPADC, 28), np.float32)
    for c in range(NCORES):
        big[NPADC * c:NPADC * (c + 1)] = res.results[c]["outp"].astype(np.float32)
    out = big[:N_NODES]
    counts = np.bincount(src, minlength=N_NODES).astype(np.float32)
    out /= np.maximum(counts, 1.0)[:, None]
    return out


# revision 3
# speedup vs baseline: 1.0100x; 1.0100x over previous
"""TensorProductConvLayer (DiffDock) Bass kernel for 8 Trainium2 cores, v3.

Metric = warm wall-clock of kernel(); the axon link moves ~70MB/s in and
~25MB/s out, so the design minimizes bytes over the link and keeps host
numpy work small. See probe.py for primitive validation.

Layout:
  - Global 128-node windows w = src>>7 (782 real). Core c owns windows
    [98c, 98c+98) = output nodes [12544c, 12544c+12544). Each window gets a
    static capacity of PW=1536 edge slots (seed-0 max occupancy is 1385);
    host scatters edges (any order within a window) into their window's
    slot range, pad slots have sh=0 so their tp contribution is exactly 0.
  - Device, per 512-edge block: MLP (edge_attr -> 48 -> 320 per-edge TP
    weights) on the PE with features on partitions; destination-node
    features fetched by indirect DMA from a row-replicated node table
    (built on device after an AllGather of 1/8 node slices) and PE-
    transposed straight into the replicated layout; TP via DVE elementwise
    + sparse stationary matmul reduction; spherical harmonics applied via
    small replicate matmuls.
  - Segment-sum: per 128-edge tile build a one-hot matrix
    sel[e, n] = (src[e]&127 == n) with a DVE is_equal against an iota
    constant, then accumulate sel^T @ tp into the window's PSUM tile over
    the window's 12 tiles; on the last tile convert to bf16 and DMA the
    128 output rows. Output = mean after a host-side divide by counts.
"""

import numpy as np
import ml_dtypes

bfl = ml_dtypes.bfloat16
f8np = ml_dtypes.float8_e4m3fn

E_TOT = 1_000_000
N_NODES = 100_000
NCORES = 8
NS = 16
NW = 98                  # windows per core
WN = 128                 # nodes per window
PW = 1536                # edge slots per window (12 tiles of 128)
TPW = PW // 128          # 12 tiles per window
NPADC = NW * WN          # 12544 output rows per core
EP = NW * PW             # 150528 edge slots per core
BLK = 512
NB = EP // BLK           # 294
NTOT = NPADC * NCORES    # 100352 rows in allgathered node table
EA_INT8 = True

_CACHE = {}
LAST_RESULTS = None


class _Results:
    """Shim mirroring BassKernelResults for the test harness."""

    def __init__(self, results):
        self.results = results
        self.exec_time_ns = None
        self.instructions_and_trace = None
        self.profile_json = None


def _get_runner(nc):
    """Build (once) a cached jitted SPMD executor for nc.

    Same execution mechanism run_bass_kernel_spmd uses under axon
    (bass2jax._bass_exec_p -> bass_exec custom call -> NEFF via PJRT on the
    8 cores), but the jitted callable, mesh, and name lists are built once
    and reused, and per-core inputs are passed as already-concatenated
    global arrays so no per-call retrace or concat copies happen.
    """
    import jax
    import numpy as _np
    from jax.experimental.shard_map import shard_map
    from jax.sharding import Mesh, PartitionSpec
    from concourse import bass2jax, mybir

    bass2jax.install_neuronx_cc_hook()
    in_names, out_names, out_avals, zero_shapes = [], [], [], []
    for alloc in nc.m.functions[0].allocations:
        if not isinstance(alloc, mybir.MemoryLocationSet):
            continue
        name = alloc.memorylocations[0].name
        if alloc.kind == "ExternalInput":
            in_names.append(name)
        elif alloc.kind == "ExternalOutput":
            out_names.append(name)
            shape = tuple(alloc.tensor_shape)
            dtype = mybir.dt.np(alloc.dtype)
            out_avals.append(jax.core.ShapedArray(shape, dtype))
            zero_shapes.append((shape, dtype))
    n_params = len(in_names)
    n_outs = len(out_names)
    names_all = tuple(in_names) + tuple(out_names)

    def _body(*args):
        outs = bass2jax._bass_exec_p.bind(
            *args,
            out_avals=tuple(out_avals),
            in_names=names_all,
            out_names=tuple(out_names),
            lowering_input_output_aliases=(),
            sim_require_finite=True,
            sim_require_nnan=True,
            nc=nc,
        )
        return tuple(outs)

    devices = jax.devices()[:NCORES]
    mesh = Mesh(_np.asarray(devices), ("core",))
    _CACHE["sharding"] = jax.sharding.NamedSharding(mesh, PartitionSpec("core"))
    in_specs = (PartitionSpec("core"),) * (n_params + n_outs)
    out_specs = (PartitionSpec("core"),) * n_outs
    donate = tuple(range(n_params, n_params + n_outs))
    jitted = jax.jit(
        shard_map(_body, mesh=mesh, in_specs=in_specs, out_specs=out_specs,
                  check_rep=False),
        donate_argnums=donate, keep_unused=True)

    def run(global_ins: dict):
        zeros = [_np.zeros((NCORES * s[0], *s[1:]), dt)
                 for s, dt in zero_shapes]
        out_arrs = jitted(*[global_ins[n] for n in in_names], *zeros)
        results = [
            {name: _np.asarray(out_arrs[i]).reshape(
                NCORES, *zero_shapes[i][0])[c] for i, name in enumerate(out_names)}
            for c in range(NCORES)
        ]
        return _Results(results)

    return run


def _build_bass():
    import concourse.bass as bass
    import concourse.bacc as bacc
    import concourse.mybir as mybir
    import concourse.tile as tile
    from concourse.masks import make_identity

    f32 = mybir.dt.float32
    bf16 = mybir.dt.bfloat16
    i32 = mybir.dt.int32
    f16 = mybir.dt.float16
    AF = mybir.ActivationFunctionType
    ea_dt = mybir.dt.uint8 if EA_INT8 else bf16

    nc = bacc.Bacc(None, target_bir_lowering=False, enable_partition_id=False,
                   num_devices=NCORES)
    ea = nc.dram_tensor("ea", [EP, 48], ea_dt, kind="ExternalInput")
    shT = nc.dram_tensor("shT", [4, EP], bf16, kind="ExternalInput")
    dstv = nc.dram_tensor("dstv", [EP, 1], i32, kind="ExternalInput")
    srcw = nc.dram_tensor("srcw", [EP, 1], bf16, kind="ExternalInput")
    nodes = nc.dram_tensor("nodes", [NPADC, 16], f16, kind="ExternalInput")
    w1d = nc.dram_tensor("w1d", [48, 48], f32, kind="ExternalInput")
    b1d = nc.dram_tensor("b1d", [48, 1], f32, kind="ExternalInput")
    w2d = nc.dram_tensor("w2d", [48, 320], f32, kind="ExternalInput")
    r16a = nc.dram_tensor("r16a", [128, 20], f32, kind="ExternalInput")
    r16b = nc.dram_tensor("r16b", [128, 20], f32, kind="ExternalInput")
    r4p = nc.dram_tensor("r4p", [64, 20], f32, kind="ExternalInput")
    rs0 = nc.dram_tensor("rs0", [4, 16], bf16, kind="ExternalInput")
    rs3 = nc.dram_tensor("rs3", [4, 12], bf16, kind="ExternalInput")
    rqd = nc.dram_tensor("rqd", [20, 12], f32, kind="ExternalInput")
    iot = nc.dram_tensor("iot", [128, 128], bf16, kind="ExternalInput")

    outp = nc.dram_tensor("outp", [NPADC, 28], bf16, kind="ExternalOutput")

    nodes_b = nc.dram_tensor("nodes_b", [NPADC, 16], f16)
    nodes_full = nc.dram_tensor("nodes_full", [NTOT, 16], f16,
                                addr_space="Shared")
    nrep = nc.dram_tensor("nrep", [NTOT, 128], f16)

    AP = bass.AP

    def dram_ap(t, off, dims):
        return AP(t, off, [list(d) for d in dims])

    with tile.TileContext(nc) as tc:
        with tc.tile_pool(name="init", bufs=2) as ip:
            nc.sync.dma_start(out=nodes_b[:, :], in_=nodes[:, :])
            nc.gpsimd.collective_compute(
                "AllGather", mybir.AluOpType.bypass,
                replica_groups=[list(range(NCORES))],
                ins=[nodes_b[:].opt()],
                outs=[nodes_full[:].opt()],
            )
            # row-replicated gather table nrep[n] = tile(x[n], 8)
            for i in range(NTOT // 1024):
                tin = ip.tile([128, 8, 16], f16, tag="tin")
                nc.sync.dma_start(
                    out=tin[:],
                    in_=dram_ap(nodes_full, 1024 * i * 16,
                                [[16, 128], [2048, 8], [1, 16]]))
                a = tin[:]
                brd = AP(a.tensor, a.offset,
                         [list(a.ap[0]), list(a.ap[1]), [0, 8], list(a.ap[2])])
                rep = ip.tile([128, 8, 8, 16], f16, tag="rep")
                nc.vector.tensor_copy(out=rep[:], in_=brd)
                nc.sync.dma_start(
                    out=dram_ap(nrep, 1024 * i * 128,
                                [[128, 128], [16384, 8], [16, 8], [1, 16]]),
                    in_=rep[:])

        with (
            tc.tile_pool(name="const", bufs=1) as cp,
            tc.tile_pool(name="sb", bufs=3) as sb,
            tc.tile_pool(name="ps", bufs=1, space="PSUM") as pp,
            tc.tile_pool(name="ps2", bufs=1, space="PSUM") as pp2,
            tc.tile_pool(name="psw", bufs=2, space="PSUM") as pw_pool,
        ):
            idn = cp.tile([128, 128], f32)
            make_identity(nc, idn[:])
            iota_sb = cp.tile([128, 128], bf16)
            nc.sync.dma_start(out=iota_sb[:], in_=iot[:, :])
            w1_sb = cp.tile([48, 48], f32)
            nc.sync.dma_start(out=w1_sb[:], in_=w1d[:, :])
            b1_sb = cp.tile([48, 1], f32)
            nc.sync.dma_start(out=b1_sb[:], in_=b1d[:, :])
            w2_sb = cp.tile([48, 320], f32)
            nc.sync.dma_start(out=w2_sb[:], in_=w2d[:, :])
            r16a_sb = cp.tile([128, 20], f32)
            nc.sync.dma_start(out=r16a_sb[:], in_=r16a[:, :])
            r16b_sb = cp.tile([128, 20], f32)
            nc.sync.dma_start(out=r16b_sb[:], in_=r16b[:, :])
            r4p_sb = cp.tile([64, 20], f32)
            nc.sync.dma_start(out=r4p_sb[:], in_=r4p[:, :])
            rs0_sb = cp.tile([4, 16], bf16)
            nc.sync.dma_start(out=rs0_sb[:], in_=rs0[:, :])
            rs3_sb = cp.tile([4, 12], bf16)
            nc.sync.dma_start(out=rs3_sb[:], in_=rs3[:, :])
            rq_sb = cp.tile([20, 12], f32)
            nc.sync.dma_start(out=rq_sb[:], in_=rqd[:, :])

            win_ps = None
            for b in range(NB):
                # ---- load ea block [128, 4, 48] ----
                ea8_sb = sb.tile([128, 4, 48],
                                 mybir.dt.uint8 if EA_INT8 else bf16, tag="ea8")
                nc.sync.dma_start(
                    out=ea8_sb[:],
                    in_=dram_ap(ea, 512 * b * 48,
                                [[48, 128], [6144, 4], [1, 48]]))
                ea_sb = sb.tile([128, 4, 48], f32, tag="ea")
                nc.scalar.activation(ea_sb[:], ea8_sb[:], AF.Copy,
                                     bias=-128.0)

                # ---- transpose to eaT [48, 512] ----
                tr_ps = pp.tile([128, 512], f32, tag="tr")
                for c in range(4):
                    nc.tensor.transpose(out=tr_ps[0:48, 128 * c:128 * (c + 1)],
                                        in_=ea_sb[:, c, :], identity=idn[:])
                eaT_sb = sb.tile([48, 512], f32, tag="eaT")
                nc.scalar.activation(eaT_sb[:], tr_ps[0:48, :], AF.Copy)

                # ---- MLP ----
                ph_ps = pp.tile([48, 512], f32, tag="ph")
                nc.tensor.matmul(ph_ps[:], lhsT=w1_sb[:], rhs=eaT_sb[:],
                                 start=True, stop=True)
                h_sb = sb.tile([48, 512], f32, tag="h")
                nc.scalar.activation(h_sb[:], ph_ps[:], AF.Relu,
                                     bias=b1_sb[:, 0:1])
                pc_ps = pp2.tile([128, 1536], f32, tag="pc")
                nc.tensor.matmul(pc_ps[0:128, 0:512], lhsT=w2_sb[:, 0:128],
                                 rhs=h_sb[:], start=True, stop=True)
                nc.tensor.matmul(pc_ps[0:128, 512:1024], lhsT=w2_sb[:, 128:256],
                                 rhs=h_sb[:], start=True, stop=True)
                nc.tensor.matmul(pc_ps[0:64, 1024:1536], lhsT=w2_sb[:, 256:320],
                                 rhs=h_sb[:], start=True, stop=True)

                # ---- gather replicated x rows, transpose to Xrep [128,512] ----
                dst_sb = sb.tile([128, 4], i32, tag="dst")
                nc.sync.dma_start(
                    out=dst_sb[:],
                    in_=dram_ap(dstv, 512 * b, [[1, 128], [128, 4]]))
                xg_sb = sb.tile([128, 4, 128], f16, tag="xg")
                for c in range(4):
                    nc.gpsimd.indirect_dma_start(
                        out=xg_sb[:, c, :], out_offset=None,
                        in_=nrep[:],
                        in_offset=bass.IndirectOffsetOnAxis(
                            ap=dst_sb[:, c:c + 1], axis=0),
                    )
                xgf_sb = sb.tile([128, 4, 128], f32, tag="xgf")
                nc.scalar.activation(xgf_sb[:], xg_sb[:], AF.Copy)
                for c in range(4):
                    nc.tensor.transpose(out=tr_ps[:, 128 * c:128 * (c + 1)],
                                        in_=xgf_sb[:, c, :], identity=idn[:])
                xr_sb = sb.tile([128, 512], f32, tag="xr")
                nc.scalar.activation(xr_sb[:], tr_ps[:], AF.Copy)

                # ---- TP elementwise + i-reduction ----
                c1_sb = sb.tile([128, 512], f32, tag="c1")
                nc.vector.tensor_tensor(out=c1_sb[:], in0=xr_sb[:],
                                        in1=pc_ps[0:128, 0:512],
                                        op=mybir.AluOpType.mult)
                c2_sb = sb.tile([128, 512], f32, tag="c2")
                nc.vector.tensor_tensor(out=c2_sb[:], in0=xr_sb[:],
                                        in1=pc_ps[0:128, 512:1024],
                                        op=mybir.AluOpType.mult)
                c3_sb = sb.tile([64, 512], f32, tag="c3")
                nc.vector.tensor_tensor(out=c3_sb[:], in0=xr_sb[0:64, :],
                                        in1=pc_ps[0:64, 1024:1536],
                                        op=mybir.AluOpType.mult)
                mix_ps = pp.tile([128, 512], f32, tag="mix")
                po = mix_ps[0:20, :]
                nc.tensor.matmul(po, lhsT=r16a_sb[:], rhs=c1_sb[:],
                                 start=True, stop=False)
                nc.tensor.matmul(po, lhsT=r16b_sb[:], rhs=c2_sb[:],
                                 start=False, stop=False)
                nc.tensor.matmul(po, lhsT=r4p_sb[:], rhs=c3_sb[:],
                                 start=False, stop=True)
                po_sb = sb.tile([20, 512], f32, tag="posb")
                nc.scalar.activation(po_sb[:], po, AF.Copy)

                # ---- spherical harmonics ----
                shT_sb = sb.tile([4, 512], bf16, tag="sh")
                nc.sync.dma_start(
                    out=shT_sb[:],
                    in_=dram_ap(shT, 512 * b, [[EP, 4], [1, 512]]))
                nc.tensor.matmul(mix_ps[32:48, :], lhsT=rs0_sb[:],
                                 rhs=shT_sb[:], start=True, stop=True)
                nc.tensor.matmul(mix_ps[64:76, :], lhsT=rs3_sb[:],
                                 rhs=shT_sb[:], start=True, stop=True)
                nc.tensor.matmul(ph_ps[0:12, :], lhsT=rq_sb[:],
                                 rhs=po_sb[:], start=True, stop=True)
                sh12_sb = sb.tile([12, 512], f32, tag="sh12")
                nc.scalar.activation(sh12_sb[:], mix_ps[64:76, :], AF.Copy)
                tpt_sb = sb.tile([16, 512], f32, tag="tpt")
                nc.vector.tensor_tensor(out=tpt_sb[:], in0=po_sb[0:16, :],
                                        in1=mix_ps[32:48, :],
                                        op=mybir.AluOpType.mult)
                tpb_sb = sb.tile([12, 512], f32, tag="tpb")
                nc.vector.tensor_tensor(out=tpb_sb[:], in0=sh12_sb[:],
                                        in1=ph_ps[0:12, :],
                                        op=mybir.AluOpType.mult)

                # ---- transpose tp to edge-major ----
                for c in range(4):
                    nc.tensor.transpose(out=tr_ps[:, 128 * c:128 * c + 16],
                                        in_=tpt_sb[:, 128 * c:128 * (c + 1)],
                                        identity=idn[0:16, 0:16])
                    nc.tensor.transpose(out=tr_ps[:, 128 * c + 16:128 * c + 28],
                                        in_=tpb_sb[:, 128 * c:128 * (c + 1)],
                                        identity=idn[0:12, 0:12])
                tpe_sb = sb.tile([128, 4, 28], f32, tag="tpe")
                for c in range(4):
                    nc.scalar.activation(tpe_sb[:, c, :],
                                         tr_ps[:, 128 * c:128 * c + 28],
                                         AF.Copy)

                # ---- windowed one-hot scatter ----
                srcw_sb = sb.tile([128, 4], bf16, tag="srcw")
                nc.sync.dma_start(
                    out=srcw_sb[:],
                    in_=dram_ap(srcw, 512 * b, [[1, 128], [128, 4]]))
                for c in range(4):
                    h = 4 * b + c
                    w, hw = divmod(h, TPW)
                    if hw == 0:
                        win_ps = pw_pool.tile([128, 28], f32, tag="win")
                    sel_sb = sb.tile([128, 128], f32, tag="sel")
                    nc.vector.tensor_tensor(
                        out=sel_sb[:],
                        in0=srcw_sb[:, c:c + 1].to_broadcast([128, 128]),
                        in1=iota_sb[:],
                        op=mybir.AluOpType.is_equal)
                    nc.tensor.matmul(win_ps[:], lhsT=sel_sb[:],
                                     rhs=tpe_sb[:, c, :],
                                     start=(hw == 0), stop=(hw == TPW - 1))
                    if hw == TPW - 1:
                        o_sb = sb.tile([128, 28], bf16, tag="ob")
                        nc.scalar.activation(o_sb[:], win_ps[:], AF.Copy)
                        nc.sync.dma_start(
                            out=dram_ap(outp, 128 * w * 28,
                                        [[28, 128], [1, 28]]),
                            in_=o_sb[:])
    nc.finalize()
    return nc


def _prep_consts(w1, b1, w2, b2, ea_scale):
    inv = np.float32(1.0 / np.sqrt(np.float32(NS)))
    w1 = np.asarray(w1, np.float32)
    b1 = np.asarray(b1, np.float32)
    w2 = np.asarray(w2, np.float32)
    b2 = np.asarray(b2, np.float32)
    assert not np.any(b2), "nonzero b2 unsupported"
    wb = w2 * inv
    p = np.arange(256)
    perm0 = (p % 16) * 16 + p // 16            # row 16j+i <- col i*16+j
    p = np.arange(64)
    perm1 = 256 + (p % 16) * 4 + p // 16       # row 16u+i <- col 256+i*4+u
    w2c = np.ascontiguousarray(wb[:, np.concatenate([perm0, perm1])])

    r16a = np.zeros((128, 20), np.float32)
    r16a[np.arange(128), np.arange(128) // 16] = 1.0
    r16b = np.zeros((128, 20), np.float32)
    r16b[np.arange(128), 8 + np.arange(128) // 16] = 1.0
    r4p = np.zeros((64, 20), np.float32)
    r4p[np.arange(64), 16 + np.arange(64) // 16] = 1.0
    rs0 = np.zeros((4, 16), np.float32)
    rs0[0, :] = 1.0
    rs3 = np.zeros((4, 12), np.float32)
    rq = np.zeros((20, 12), np.float32)
    for u in range(4):
        for m in range(3):
            rs3[1 + m, 3 * u + m] = 1.0
            rq[16 + u, 3 * u + m] = 1.0
    iota = np.broadcast_to(np.arange(128, dtype=np.float32), (128, 128))
    w1s = np.ascontiguousarray(w1 / ea_scale)
    return {"w1d": w1s, "b1d": b1.reshape(48, 1).astype(np.float32),
            "w2d": w2c, "r16a": r16a, "r16b": r16b,
            "r4p": r4p, "rs0": rs0.astype(bfl),
            "rs3": rs3.astype(bfl), "rqd": rq,
            "iot": np.ascontiguousarray(iota).astype(bfl)}


def kernel(node_attr, edge_index, edge_attr, edge_sh, w1, b1, w2, b2):
    global LAST_RESULTS
    from concourse.bass_utils import run_bass_kernel_spmd

    src = np.asarray(edge_index[0]).astype(np.int32, copy=False)
    dst = np.asarray(edge_index[1]).astype(np.int32, copy=False)
    edge_attr = np.asarray(edge_attr, np.float32)
    edge_sh = np.asarray(edge_sh, np.float32)
    node_attr = np.asarray(node_attr, np.float32)

    ea_scale = np.float32(127.0) / np.float32(np.abs(edge_attr).max())
    consts = _prep_consts(w1, b1, w2, b2, ea_scale)

    wg = src >> 7                                  # global window id
    order = np.argsort(wg, kind="stable")
    wcnt = np.bincount(wg, minlength=NW * NCORES)
    assert wcnt.max() <= PW, f"window overflow: {wcnt.max()} > {PW}"
    wstart = np.zeros(NW * NCORES + 1, np.int64)
    wstart[1:] = np.cumsum(wcnt)
    ws = wg[order]
    rank = np.arange(E_TOT, dtype=np.int64) - wstart[ws]
    slot = (ws // NW) * EP + (ws % NW) * PW + rank   # into [8*EP]

    ea_all = (edge_attr * ea_scale + np.float32(128.5)).astype(np.uint8)
    ea_pad = np.zeros((NCORES * EP, 48), np.uint8)
    ea_pad[slot] = ea_all[order]
    if "nc" not in _CACHE:
        _CACHE["nc"] = _build_bass()
    if "runner" not in _CACHE:
        _CACHE["runner"] = _get_runner(_CACHE["nc"])
    import jax as _jax
    ea_dev = _jax.device_put(ea_pad, _CACHE["sharding"])  # async; overlaps below
    sh_pad = np.zeros((NCORES * EP, 4), bfl)
    sh_pad[slot] = edge_sh[:, :4].astype(bfl)[order]
    shT_pad = np.ascontiguousarray(
        sh_pad.reshape(NCORES, EP, 4).transpose(0, 2, 1))
    dst_pad = np.zeros((NCORES * EP, 1), np.int32)
    dst_pad[slot, 0] = dst[order]
    srcw_pad = np.zeros((NCORES * EP, 1), bfl)
    srcw_pad[slot, 0] = (src[order] & 127).astype(bfl)
    nodes_pad = np.zeros((NCORES, NPADC, 16), np.float16)
    flat = nodes_pad.reshape(NCORES * NPADC, 16)
    flat[:N_NODES] = node_attr.astype(np.float16)

    # global (concatenated-over-cores) layout for the cached SPMD runner
    global_ins = {
        "ea": ea_dev,
        "shT": shT_pad.reshape(NCORES * 4, EP),
        "dstv": dst_pad,
        "srcw": srcw_pad,
        "nodes": nodes_pad.reshape(NCORES * NPADC, 16),
    }
    for k, v in consts.items():
        global_ins[k] = np.ascontiguousarray(
            np.broadcast_to(v, (NCORES, *v.shape))).reshape(
                NCORES * v.shape[0], *v.shape[1:])

    res = _CACHE["runner"](global_ins)
    LAST_RESULTS = res

    big = np.empty((NCORES * NPADC, 28), np.float32)
    for c in range(NCORES):
        big[NPADC * c:NPADC * (c + 1)] = res.results[c]["outp"].astype(np.float32)
    out = big[:N_NODES]
    counts = np.bincount(src, minlength=N_NODES).astype(np.float32)
    out /= np.maximum(counts, 1.0)[:, None]
    return out
